# revision 20
# baseline (speedup 1.0000x reference)
"""Trainium2 Bass kernel for the CGC (Customized Gate Control) MoE routing module.

Contract: kernel(**inputs) takes the FULL unsharded inputs (numpy/jax arrays)
and returns the FULL output [5, 16384, 256] float32.

Strategy (v2):
  - Data-parallel over batch across 8 NeuronCores (2048 rows/core).
  - Host prep: per-core x slices fed pre-transposed [DIN, B_c]; weights
    replicated, packed [DIN, E*H], cast to bf16. PSUM stays fp32.
  - Expert biases enter PSUM via 2-way row-tiled (tile_position) K=32 one-hot
    matmul volleys - two concurrent strip matmuls cost ~one matmul.
  - PSUM organized as three 2-bank tiles per 128-row tile (spec d0|d1,
    spec d2|d3, shared s01|s23). Each tile has exactly ONE reader: a wide
    ScalarE relu eviction to bf16 SBUF, so banks turn over fast and the PE
    never waits on stragglers.
  - The gated combine runs from the relu'd bf16 SBUF copies: per-partition
    gate scalars via tensor_scalar (4x mode) inits and scalar_tensor_tensor
    MACs, split across Vector and GpSimd; out_sh accumulates in two parallel
    chains merged at the end.
  - Gate matmuls of group g+1 are woven into group g's expert matmul stream
    (their LDWEIGHTS hide under the 512-column expert matmuls); softmax is
    batched per group.
  - The double-softmax mask is known from sim_domain at trace time; masked
    shared-expert terms are not emitted (kernel is compile-specialized).
  - Output is written bf16 over HWDGE (sync) DMA; host casts to f32.
"""

import sys

sys.path.insert(0, "/opt/trn_rl_repo")

import numpy as np

D_NUM = 4
N_ES = 2
N_SH = 4
DIN = 512
H = 256
B = 16384
N_CORES = 8
BC = B // N_CORES          # 2048 rows per core
KC = DIN // 128            # 4 contraction chunks
GRP = 8                    # batch tiles (of 128 rows) per group
NG = BC // (128 * GRP)     # groups per core

# bias4 column layout: [spec 512 | shared 512 | gate-dom 192 | gsh 96]
GB_GS = D_NUM * GRP * 6                  # 192: gate-bank dom region size
NGB = GB_GS + GRP * 12                   # 288: full gate bank
OFF_SHB = 512
OFF_GBD = 1024
OFF_GBS = OFF_GBD + GB_GS
NBIAS = OFF_GBS + GRP * 12               # 1312

_BUILD_CACHE = {}
_DUAL = None


def _get_dual():
    """Register the DUAL_SCALE_ADD custom DVE op: out = in0*s0 + in1*s1
    (two gated combine terms in one Vector instruction). Idempotent."""
    global _DUAL
    if _DUAL is not None:
        return _DUAL
    from concourse import dve_ops
    from concourse.dve_spec import Spec, Src0, Src1, C0, C1, lower, _has_src1
    from concourse.dve_uop import DveOpSpec

    name = "DUAL_SCALE_ADD_ANT"
    for o in dve_ops.OPS:
        if o.name == name:
            _DUAL = o
            return o
    spec = Spec(
        body=Src0 * C0 + Src1 * C1,
        reference=lambda in0, in1, s0, s1, imm2: (
            in0.astype(np.float32) * s0 + in1.astype(np.float32) * s1
        ),
    )
    row = max(dve_ops._SUB_OPCODE_FOR_NAME.values()) + 1
    assert row < 0x20
    dve_ops._SUB_OPCODE_FOR_NAME[name] = row
    shas = {}
    for ver in ("v3", "v4"):
        tmp = DveOpSpec(name=name, opcode=row, uops=lower(spec, ver=ver),
                        rd1_en=_has_src1(spec))
        shas[ver] = tmp.sha(ver)
    op = dve_ops.DveOp(name, spec, subdim=False, uops_sha=shas)
    dve_ops.OPS.append(op)
    dve_ops.CUSTOM_DVE_SPECS[name] = spec
    _DUAL = op
    return op


def _build(allowed):
    """Trace + compile the per-core kernel, specialized on the allowed
    shared-expert sets (from sim_domain)."""
    import concourse.bacc as bacc
    import concourse.bass as bass
    import concourse.mybir as mybir
    import concourse.tile as tile

    DUAL = _get_dual()
    f32 = mybir.dt.float32
    bf16 = mybir.dt.bfloat16
    Alu = mybir.AluOpType
    Act = mybir.ActivationFunctionType
    Ax = mybir.AxisListType

    nc = bacc.Bacc(None, target_bir_lowering=False, debug=False)

    xt = nc.declare_dram_parameter("xt", [5, DIN, BC], bf16, isOutput=False)
    wsp = nc.declare_dram_parameter("wsp", [D_NUM, DIN, N_ES * H], bf16, isOutput=False)
    wsh = nc.declare_dram_parameter("wsh", [DIN, N_SH * H], bf16, isOutput=False)
    wg = nc.declare_dram_parameter("wg", [DIN, D_NUM * 6], bf16, isOutput=False)
    wgs = nc.declare_dram_parameter("wgs", [DIN, 12], bf16, isOutput=False)
    bias4 = nc.declare_dram_parameter("bias4", [128, NBIAS], bf16, isOutput=False)
    ones128 = nc.declare_dram_parameter("ones128", [128, 128], bf16, isOutput=False)
    bmask = nc.declare_dram_parameter("bmask", [128, D_NUM, GRP, 6], f32, isOutput=False)
    out = nc.declare_dram_parameter("out", [5, BC, H], bf16, isOutput=True)

    with tile.TileContext(nc) as tc:
        with (
            tc.tile_pool(name="wpool", bufs=1) as wp,
            tc.tile_pool(name="xpool", bufs=2) as xp,
            tc.tile_pool(name="ogpool", bufs=2) as ogp,
            tc.tile_pool(name="smpool", bufs=3) as sp,
            tc.tile_pool(name="rpool", bufs=7) as rp,
            tc.tile_pool(name="scrpool", bufs=4) as scp,
            tc.tile_pool(name="pbig", bufs=3, space=bass.MemorySpace.PSUM) as pb,
            tc.tile_pool(name="pgate", bufs=1, space=bass.MemorySpace.PSUM) as pg,
        ):
            # ---- small weights first so the gate phase starts ASAP ----
            wg_sb = wp.tile([128, KC, D_NUM * 6], bf16, tag="wg")
            nc.sync.dma_start(wg_sb[:], wg.rearrange("(c p) n -> p c n", p=128))
            wgs_sb = wp.tile([128, KC, 12], bf16, tag="wgs")
            nc.sync.dma_start(wgs_sb[:], wgs.rearrange("(c p) n -> p c n", p=128))
            bias_sb = wp.tile([128, NBIAS], bf16, tag="bias")
            nc.sync.dma_start(bias_sb[:], bias4[:])
            ones_sb = wp.tile([128, 128], bf16, tag="ones")
            nc.sync.dma_start(ones_sb[:], ones128[:])
            bmask_sb = wp.tile([128, D_NUM, GRP, 6], f32, tag="bmask")
            nc.sync.dma_start(bmask_sb[:], bmask[:])

            # x of group 0 (per-domain tiles) interleaved with expert weights
            # so domain-0 compute starts after ~1.5 MB of DMA, not ~9 MB.
            # Separate tiles per slice i -> per-domain DMA dependencies.
            wsp_sb = wp.tile([128, D_NUM, KC, N_ES * H], bf16, tag="wsp")
            wsh_sb = wp.tile([128, KC, N_SH * H], bf16, tag="wsh")

            def alloc_xtg(g):
                # xs slice (i=4) first: the gsh gate matmuls and shared experts
                # unblock early, so the group softmax isn't the critical path.
                j0 = g * (GRP * 128)
                xtg = [None] * 5
                for i in (4, 0, 1, 2, 3):
                    xi = xp.tile([128, KC, GRP * 128], bf16, tag=f"xtg{i}",
                                 name=f"xtg{g}_{i}")
                    nc.sync.dma_start(
                        xi[:], xt[i, :, j0 : j0 + GRP * 128].rearrange(
                            "(c p) j -> p c j", p=128))
                    xtg[i] = xi
                    if g == 0:
                        if i < D_NUM:
                            nc.sync.dma_start(
                                wsp_sb[:, i], wsp[i].rearrange("(c p) n -> p c n", p=128))
                        else:
                            nc.sync.dma_start(
                                wsh_sb[:], wsh.rearrange("(c p) n -> p c n", p=128))
                return xtg

            def gate_mm_emitters(g, xtg, gbank):
                """List of closures, one per gate matmul (bias first)."""
                gbd, gbs = gbank
                ems = [lambda: nc.tensor.matmul(
                    gbd[:], ones_sb[0:32, :], bias_sb[0:32, OFF_GBD : OFF_GBD + GB_GS],
                    start=True, stop=False, skip_group_check=True),
                       lambda: nc.tensor.matmul(
                    gbs[:], ones_sb[0:32, :], bias_sb[0:32, OFF_GBS : NBIAS],
                    start=True, stop=False, skip_group_check=True)]
                def dom(d, t, c):
                    o0 = (d * GRP + t) * 6
                    ems.append(lambda: nc.tensor.matmul(
                        gbd[:, o0 : o0 + 6],
                        xtg[d][:, c, t * 128 : (t + 1) * 128],
                        wg_sb[:, c, 6 * d : 6 * d + 6],
                        start=False, stop=False, skip_group_check=True))
                def gsh(t, c, last):
                    o1 = t * 12
                    ems.append(lambda: nc.tensor.matmul(
                        gbs[:, o1 : o1 + 12],
                        xtg[4][:, c, t * 128 : (t + 1) * 128],
                        wgs_sb[:, c, :],
                        start=False, stop=last, skip_group_check=True))
                if g == 0:
                    # gsh first (xs slice lands first), then domain-major
                    for t in range(GRP):
                        for c in range(KC):
                            gsh(t, c, t == GRP - 1 and c == KC - 1)
                    for d in range(D_NUM):
                        for t in range(GRP):
                            for c in range(KC):
                                dom(d, t, c)
                else:
                    for t in range(GRP):
                        for d in range(D_NUM):
                            for c in range(KC):
                                dom(d, t, c)
                        for c in range(KC):
                            gsh(t, c, t == GRP - 1 and c == KC - 1)
                return ems

            def emit_softmax(g, gbank):
                gbd, gbs = gbank
                gview = gbd.rearrange("p (d t s) -> p d t s", d=D_NUM, t=GRP)
                e1 = sp.tile([128, D_NUM, GRP, 6], f32, tag="e1", name=f"e1_{g}")
                nc.scalar.activation(e1[:], gview, Act.Exp)
                s1 = sp.tile([128, D_NUM, GRP], f32, tag="s1", name=f"s1_{g}")
                nc.vector.tensor_reduce(s1[:], e1[:], axis=Ax.X, op=Alu.add)
                r1 = sp.tile([128, D_NUM, GRP], f32, tag="r1", name=f"r1_{g}")
                nc.vector.reciprocal(r1[:], s1[:])
                gn = sp.tile([128, D_NUM, GRP, 6], f32, tag="gn", name=f"gn_{g}")
                nc.vector.tensor_tensor(
                    gn[:], e1[:], r1[:, :, :, None].to_broadcast([128, D_NUM, GRP, 6]),
                    Alu.mult)
                e2 = sp.tile([128, D_NUM, GRP, 6], f32, tag="e2", name=f"e2_{g}")
                nc.scalar.activation(e2[:], gn[:], Act.Exp)
                e2m = sp.tile([128, D_NUM, GRP, 6], f32, tag="e2m", name=f"e2m_{g}")
                nc.vector.tensor_tensor(e2m[:], e2[:], bmask_sb[:], Alu.mult)
                s2 = sp.tile([128, D_NUM, GRP], f32, tag="s2", name=f"s2_{g}")
                nc.vector.tensor_reduce(s2[:], e2m[:], axis=Ax.X, op=Alu.add)
                r2 = sp.tile([128, D_NUM, GRP], f32, tag="r2", name=f"r2_{g}")
                nc.vector.reciprocal(r2[:], s2[:])
                g2 = sp.tile([128, D_NUM, GRP, 6], f32, tag="g2", name=f"g2_{g}")
                nc.vector.tensor_tensor(
                    g2[:], e2m[:], r2[:, :, :, None].to_broadcast([128, D_NUM, GRP, 6]),
                    Alu.mult)

                gsview = gbs.rearrange("p (t s) -> p t s", t=GRP)
                egs = sp.tile([128, GRP, 12], f32, tag="egs", name=f"egs{g}")
                nc.scalar.activation(egs[:], gsview, Act.Exp)
                sgs = sp.tile([128, GRP], f32, tag="sgs", name=f"sgs{g}")
                nc.vector.tensor_reduce(sgs[:], egs[:], axis=Ax.X, op=Alu.add)
                rgs = sp.tile([128, GRP], f32, tag="rgs", name=f"rgs{g}")
                nc.vector.reciprocal(rgs[:], sgs[:])
                gs = sp.tile([128, GRP, 12], f32, tag="gs", name=f"gs{g}")
                nc.vector.tensor_tensor(
                    gs[:], egs[:], rgs[:, :, None].to_broadcast([128, GRP, 12]), Alu.mult)
                return g2, gs

            pending = [None]

            def flush_pending():
                if pending[0] is not None:
                    em, pending[0] = pending[0], None
                    em()

            def emit_tile(g, t, xtg, og, g2, gs, gate_block):
                """Expert matmuls + evictions + combines for one 128-row tile.
                gate_block: next-group gate-MM emitters woven into the stream."""
                j0 = g * (GRP * 128)
                gi = iter(gate_block)

                def weave(n):
                    for _ in range(n):
                        em = next(gi, None)
                        if em is not None:
                            em()

                # --- spec experts: two 2-bank psum tiles, bias via ONE 4-strip
                # volley (4 concurrent K=32 row-tiled matmuls)
                ps_pair = [
                    pb.tile([128, 1024], f32, tag="pb", name=f"ps{g}_{t}_0"),
                    pb.tile([128, 1024], f32, tag="pb", name=f"ps{g}_{t}_1"),
                ]
                for d in range(D_NUM):
                    rr = 32 * d
                    nc.tensor.matmul(ps_pair[d // 2][:, 512 * (d % 2) : 512 * (d % 2) + 512],
                                     ones_sb[rr : rr + 32, :],
                                     bias_sb[rr : rr + 32, 0:512],
                                     start=True, stop=False, skip_group_check=True,
                                     tile_position=(rr, 0))
                for half in range(2):
                    ps = ps_pair[half]
                    for dd in range(2):
                        d = 2 * half + dd
                        o = 512 * dd
                        for c in range(KC):
                            nc.tensor.matmul(ps[:, o : o + 512],
                                             xtg[d][:, c, t * 128 : (t + 1) * 128],
                                             wsp_sb[:, d, c, :], start=False,
                                             stop=(c == KC - 1), skip_group_check=True)
                            weave(1)

                # --- shared experts: one 2-bank psum tile
                ps_s = pb.tile([128, 1024], f32, tag="pb", name=f"pss{g}_{t}")
                nc.tensor.matmul(ps_s[:, 0:512], ones_sb[0:32, :],
                                 bias_sb[0:32, OFF_SHB : OFF_SHB + 512],
                                 start=True, stop=False, skip_group_check=True,
                                 tile_position=(0, 0))
                nc.tensor.matmul(ps_s[:, 512:1024], ones_sb[32:64, :],
                                 bias_sb[32:64, OFF_SHB : OFF_SHB + 512],
                                 start=True, stop=False, skip_group_check=True,
                                 tile_position=(32, 0))
                for c in range(KC):
                    nc.tensor.matmul(ps_s[:, 0:512], xtg[4][:, c, t * 128 : (t + 1) * 128],
                                     wsh_sb[:, c, 0:512], start=False,
                                     stop=(c == KC - 1), skip_group_check=True)
                    nc.tensor.matmul(ps_s[:, 512:1024], xtg[4][:, c, t * 128 : (t + 1) * 128],
                                     wsh_sb[:, c, 512:1024], start=False,
                                     stop=(c == KC - 1), skip_group_check=True)
                    weave(1)
                weave(len(gate_block))  # flush leftovers

                # --- evictions: ONE wide relu act per psum tile -> bf16 SBUF
                r01 = rp.tile([128, 1024], bf16, tag="r", name=f"r01_{g}_{t}")
                nc.scalar.activation(r01[:], ps_pair[0][:], Act.Relu)
                r23 = rp.tile([128, 1024], bf16, tag="r", name=f"r23_{g}_{t}")
                nc.scalar.activation(r23[:], ps_pair[1][:], Act.Relu)
                rsh = rp.tile([128, 1024], bf16, tag="r", name=f"rsh_{g}_{t}")
                nc.scalar.activation(rsh[:], ps_s[:], Act.Relu)
                # previous tile's ScalarE products run AFTER this tile's
                # evictions so PSUM banks always turn over promptly
                flush_pending()

                def rspec(d, e):
                    rt = r01 if d < 2 else r23
                    o = (d % 2) * 512 + e * 256
                    return rt[:, o : o + 256]

                def rshared(s):
                    return rsh[:, s * 256 : (s + 1) * 256]

                # --- combine: DUAL_SCALE_ADD fuses two gated terms per Vector
                # instruction; GpSimd does the tensor_tensor glue adds.
                def dual(out_ap, a_src, a_scale, b_src, b_scale):
                    nc.vector._custom_dve(DUAL, out=out_ap, in0=a_src, in1=b_src,
                                          s0=a_scale, s1=b_scale)

                # In the drain tail (last tiles of the last group) Vector is the
                # only backlogged engine: route products to Scalar + adds to
                # GpSimd there instead of Vector duals.
                tail = g == NG - 1 and t >= GRP - 2

                def pair(out_ap, a_src, a_scale, b_src, b_scale, slot, glue_gp):
                    """out = a*sa + b*sb via one Vector dual, or (in the tail)
                    two ScalarE products + one add."""
                    if not tail:
                        dual(out_ap, a_src, a_scale, b_src, b_scale)
                    else:
                        nc.scalar.activation(out_ap, a_src, Act.Copy, scale=a_scale)
                        nc.scalar.activation(scr[:, slot], b_src, Act.Copy,
                                             scale=b_scale)
                        if glue_gp:
                            nc.gpsimd.tensor_tensor(out_ap, out_ap, scr[:, slot],
                                                    Alu.add)
                        else:
                            nc.vector.tensor_tensor(out_ap, out_ap, scr[:, slot],
                                                    Alu.add)

                scr = scp.tile([128, 8, 256], bf16, tag="scr", name=f"scr_{g}_{t}")
                for d in range(D_NUM):
                    og_d = og[:, d, t, :]
                    pair(og_d, rspec(d, 0), g2[:, d, t, 0:1],
                         rspec(d, 1), g2[:, d, t, 1:2], 6 + d % 2, d % 2 == 0)
                    al = allowed[d]
                    i = 0
                    while i + 1 < len(al):
                        sa, sb = al[i], al[i + 1]
                        dual(scr[:, 4], rshared(sa), g2[:, d, t, 2 + sa : 3 + sa],
                             rshared(sb), g2[:, d, t, 2 + sb : 3 + sb])
                        nc.gpsimd.tensor_tensor(og_d, og_d, scr[:, 4], Alu.add)
                        i += 2
                    if i < len(al):
                        s = al[i]
                        nc.vector.scalar_tensor_tensor(
                            og_d, rshared(s), g2[:, d, t, 2 + s : 3 + s], og_d,
                            Alu.mult, Alu.add)

                # og_s: 12 terms as 6 duals; chain A accumulates on Vector,
                # chain B glue on GpSimd, merged by GpSimd into the og slot.
                og_s = og[:, 4, t, :]
                pair(og_s, rspec(0, 0), gs[:, t, 0:1], rspec(0, 1), gs[:, t, 1:2],
                     3, False)
                dual(scr[:, 0], rspec(1, 0), gs[:, t, 2:3], rspec(1, 1), gs[:, t, 3:4])
                dual(scr[:, 1], rshared(0), gs[:, t, 8:9], rshared(1), gs[:, t, 9:10])
                nc.gpsimd.tensor_tensor(scr[:, 0], scr[:, 0], scr[:, 1], Alu.add)
                nc.vector.tensor_tensor(og_s, og_s, scr[:, 0], Alu.add)
                # chain B (deferred one tile): six ScalarE scaled-copy products
                # + GpSimd/Vector glue, then the output DMA for this tile.
                def chain_b():
                    sc2 = scp.tile([128, 6, 256], bf16, tag="sc2", name=f"sc2_{g}_{t}")
                    for i, (src, k) in enumerate(
                            ((rspec(2, 0), 4), (rspec(2, 1), 5), (rspec(3, 0), 6),
                             (rspec(3, 1), 7), (rshared(2), 10), (rshared(3), 11))):
                        nc.scalar.activation(sc2[:, i], src, Act.Copy,
                                             scale=gs[:, t, k : k + 1])
                    nc.gpsimd.tensor_tensor(sc2[:, 0], sc2[:, 0], sc2[:, 1], Alu.add)
                    nc.gpsimd.tensor_tensor(sc2[:, 2], sc2[:, 2], sc2[:, 3], Alu.add)
                    nc.gpsimd.tensor_tensor(sc2[:, 4], sc2[:, 4], sc2[:, 5], Alu.add)
                    nc.vector.tensor_tensor(sc2[:, 0], sc2[:, 0], sc2[:, 2], Alu.add)
                    nc.vector.tensor_tensor(sc2[:, 0], sc2[:, 0], sc2[:, 4], Alu.add)
                    nc.vector.tensor_tensor(og_s, og_s, sc2[:, 0], Alu.add)

                    r0_ = j0 + t * 128
                    nc.sync.dma_start(
                        out[:, r0_ : r0_ + 128, :].rearrange("i p h -> p i h"),
                        og[:, :, t, :])

                pending[0] = chain_b

            # ---- software pipeline over groups ----
            xtg_cur = alloc_xtg(0)
            gb0 = pg.tile([128, NGB], f32, tag="pg", name="gb0")
            gbank_cur = (gb0[:, 0:GB_GS], gb0[:, GB_GS:NGB])
            for em in gate_mm_emitters(0, xtg_cur, gbank_cur):
                em()
            sm_cur = emit_softmax(0, gbank_cur)

            for g in range(NG):
                og = ogp.tile([128, 5, GRP, H], bf16, tag="og", name=f"og{g}")
                if g + 1 < NG:
                    xtg_next = alloc_xtg(g + 1)
                    gbn = pg.tile([128, NGB], f32, tag="pg", name=f"gb{g+1}")
                    gbank_next = (gbn[:, 0:GB_GS], gbn[:, GB_GS:NGB])
                    ems = gate_mm_emitters(g + 1, xtg_next, gbank_next)
                    # weave gate MMs over tiles 1..5 only, so the next group's
                    # softmax can be emitted after tile 5 and its ops queue
                    # AHEAD of tiles 6-7's combine work on Vector/Scalar
                    # (otherwise the group boundary serializes on the Vector
                    # backlog and the PE stalls ~12 us).
                    nblk = 5
                    per = (len(ems) + nblk - 1) // nblk
                    blocks = ([[]] + [ems[i * per : (i + 1) * per] for i in range(nblk)]
                              + [[] for _ in range(GRP - 1 - nblk)])
                else:
                    blocks = [[] for _ in range(GRP)]
                sm_next = None
                for t in range(GRP):
                    emit_tile(g, t, xtg_cur, og, sm_cur[0], sm_cur[1], blocks[t])
                    if g + 1 < NG and t == nblk:
                        sm_next = emit_softmax(g + 1, gbank_next)
                if g + 1 < NG:
                    sm_cur = sm_next
                    xtg_cur = xtg_next
            flush_pending()

    nc.compile()
    return nc


def _prep_inputs(inputs):
    """Host-side shard + relayout. Returns (in_maps, allowed)."""
    import ml_dtypes
    bf16_np = ml_dtypes.bfloat16

    x_list = np.asarray(inputs["x_list"], dtype=np.float32)
    sim_domain = np.asarray(inputs["sim_domain"])
    W_spec = np.asarray(inputs["W_spec"], dtype=np.float32)
    b_spec = np.asarray(inputs["b_spec"], dtype=np.float32)
    W_sh = np.asarray(inputs["W_sh"], dtype=np.float32)
    b_sh = np.asarray(inputs["b_sh"], dtype=np.float32)
    W_gate = np.asarray(inputs["W_gate"], dtype=np.float32)
    b_gate = np.asarray(inputs["b_gate"], dtype=np.float32)
    W_gate_sh = np.asarray(inputs["W_gate_sh"], dtype=np.float32)
    b_gate_sh = np.asarray(inputs["b_gate_sh"], dtype=np.float32)

    mem = (sim_domain[:, :, None] == np.arange(D_NUM)[None, None, :]).any(axis=1)
    allowed = tuple(tuple(int(s) for s in range(N_SH) if mem[d, s]) for d in range(D_NUM))

    wsp = np.ascontiguousarray(
        W_spec.transpose(0, 2, 1, 3).reshape(D_NUM, DIN, N_ES * H)
    ).astype(bf16_np)
    wsh = np.ascontiguousarray(W_sh.transpose(1, 0, 2).reshape(DIN, N_SH * H)).astype(bf16_np)
    wg = np.ascontiguousarray(W_gate.transpose(1, 0, 2).reshape(DIN, D_NUM * 6)).astype(bf16_np)
    wgs = np.ascontiguousarray(W_gate_sh).astype(bf16_np)

    bias4 = np.zeros((128, NBIAS), np.float32)
    for d in range(D_NUM):
        bias4[32 * d, 0:512] = b_spec[d].reshape(512)
    bias4[0, OFF_SHB : OFF_SHB + 512] = b_sh[0:2].reshape(512)
    bias4[32, OFF_SHB : OFF_SHB + 512] = b_sh[2:4].reshape(512)
    bias4[0, OFF_GBD : OFF_GBD + GB_GS] = np.repeat(
        b_gate[:, None, :], GRP, axis=1).reshape(-1)
    bias4[0, OFF_GBS:NBIAS] = np.tile(b_gate_sh, GRP)
    bias4 = bias4.astype(bf16_np)

    ones128 = np.zeros((128, 128), np.float32)
    for s in range(4):
        ones128[32 * s] = 1.0
    ones128 = ones128.astype(bf16_np)

    bmask_row = np.ones((D_NUM, 6), np.float32)
    bmask_row[:, N_ES:] = mem.astype(np.float32)
    bmask = np.broadcast_to(
        np.repeat(bmask_row[None, :, None, :], GRP, axis=2), (128, D_NUM, GRP, 6)
    ).copy()

    shared = {"wsp": wsp, "wsh": wsh, "wg": wg, "wgs": wgs,
              "bias4": bias4, "ones128": ones128, "bmask": bmask}
    in_maps = []
    for c in range(N_CORES):
        sl = x_list[:, c * BC : (c + 1) * BC, :]
        xt_c = np.ascontiguousarray(sl.transpose(0, 2, 1)).astype(bf16_np)
        in_maps.append({"xt": xt_c, **shared})
    return in_maps, allowed


def _run(inputs, trace=False, trace_kwargs=None):
    from concourse.bass_utils import run_bass_kernel_spmd

    in_maps, allowed = _prep_inputs(inputs)
    key = allowed
    if key not in _BUILD_CACHE:
        _BUILD_CACHE[key] = _build(allowed)
    nc = _BUILD_CACHE[key]

    kw = {}
    if trace:
        kw["trace"] = True
        if trace_kwargs:
            kw.update(trace_kwargs)
    res = run_bass_kernel_spmd(nc, in_maps, list(range(N_CORES)), **kw)
    full = np.empty((5, B, H), np.float32)
    for c in range(N_CORES):
        full[:, c * BC : (c + 1) * BC, :] = np.asarray(
            res.results[c]["out"], dtype=np.float32)
    return full, res


def kernel(**inputs):
    full, _ = _run(inputs)
    return full


# revision 26
# speedup vs baseline: 1.0725x; 1.0725x over previous
"""Trainium2 Bass kernel for the CGC (Customized Gate Control) MoE routing module.

Contract: kernel(**inputs) takes the FULL unsharded inputs (numpy/jax arrays)
and returns the FULL output [5, 16384, 256] float32.

Strategy (v2):
  - Data-parallel over batch across 8 NeuronCores (2048 rows/core).
  - Host prep: per-core x slices fed pre-transposed [DIN, B_c]; weights
    replicated, packed [DIN, E*H], cast to bf16. PSUM stays fp32.
  - Expert biases enter PSUM via 2-way row-tiled (tile_position) K=32 one-hot
    matmul volleys - two concurrent strip matmuls cost ~one matmul.
  - PSUM organized as three 2-bank tiles per 128-row tile (spec d0|d1,
    spec d2|d3, shared s01|s23). Each tile has exactly ONE reader: a wide
    ScalarE relu eviction to bf16 SBUF, so banks turn over fast and the PE
    never waits on stragglers.
  - The gated combine runs from the relu'd bf16 SBUF copies: per-partition
    gate scalars via tensor_scalar (4x mode) inits and scalar_tensor_tensor
    MACs, split across Vector and GpSimd; out_sh accumulates in two parallel
    chains merged at the end.
  - Gate matmuls of group g+1 are woven into group g's expert matmul stream
    (their LDWEIGHTS hide under the 512-column expert matmuls); softmax is
    batched per group.
  - The double-softmax mask is known from sim_domain at trace time; masked
    shared-expert terms are not emitted (kernel is compile-specialized).
  - Output is written bf16 over HWDGE (sync) DMA; host casts to f32.
"""

import sys

sys.path.insert(0, "/opt/trn_rl_repo")

import numpy as np

D_NUM = 4
N_ES = 2
N_SH = 4
DIN = 512
H = 256
B = 16384
N_CORES = 8
BC = B // N_CORES          # 2048 rows per core
KC = DIN // 128            # 4 contraction chunks
GRP = 8                    # batch tiles (of 128 rows) per group
NG = BC // (128 * GRP)     # groups per core

# bias4 column layout: [spec 512 | shared 512 | gate-dom 192 | gsh 96]
GB_GS = D_NUM * GRP * 6                  # 192: gate-bank dom region size
NGB = GB_GS + GRP * 12                   # 288: full gate bank
OFF_SHB = 512
OFF_GBD = 1024
OFF_GBS = OFF_GBD + GB_GS
NBIAS = OFF_GBS + GRP * 12               # 1312

_BUILD_CACHE = {}
_DUAL = None
_EXP3M = None

# minimax quadratic correction: e^g ~= 1 + g*(EA*g^2 + EB*g + EC) on [0,1]
# (max rel err 3.7e-4); used for the SECOND softmax exp, whose input is the
# output of the first softmax and hence in [0,1].
EA = 0.266761673
EB = 0.441609546
EC = 1.008912084


def _get_exp3m():
    """Register EXP3M_ANT: out = in1 * (1 + in0*((s0*in0 + s1)*in0 + imm2))
    = bmask * exp-approx(g). Runs the second-softmax exp (fused with the
    similarity mask) on the Vector engine, so no ScalarE op ever depends on
    the Vector backlog (that dependency head-of-line-blocked the PSUM
    evictions behind it and stalled the PE at group boundaries)."""
    global _EXP3M
    if _EXP3M is not None:
        return _EXP3M
    from concourse import dve_ops
    from concourse.dve_spec import Spec, Src0, Src1, C0, C1, C2, lower, _has_src1
    from concourse.dve_uop import DveOpSpec

    name = "EXP3M_ANT"
    for o in dve_ops.OPS:
        if o.name == name:
            _EXP3M = o
            return o
    spec = Spec(
        body=Src0 * ((C0 * Src0 + C1) * Src0 + C2) * Src1 + Src1,
        reference=lambda in0, in1, s0, s1, imm2: (
            in0.astype(np.float32)
            * ((s0 * in0 + s1) * in0.astype(np.float32) + imm2)
            * in1
            + in1
        ),
    )
    row = max(dve_ops._SUB_OPCODE_FOR_NAME.values()) + 1
    assert row < 0x20
    dve_ops._SUB_OPCODE_FOR_NAME[name] = row
    shas = {}
    for ver in ("v3", "v4"):
        tmp = DveOpSpec(name=name, opcode=row, uops=lower(spec, ver=ver),
                        rd1_en=_has_src1(spec))
        shas[ver] = tmp.sha(ver)
    op = dve_ops.DveOp(name, spec, subdim=False, uops_sha=shas)
    dve_ops.OPS.append(op)
    dve_ops.CUSTOM_DVE_SPECS[name] = spec
    _EXP3M = op
    return op


def _get_dual():
    """Register the DUAL_SCALE_ADD custom DVE op: out = in0*s0 + in1*s1
    (two gated combine terms in one Vector instruction). Idempotent."""
    global _DUAL
    if _DUAL is not None:
        return _DUAL
    from concourse import dve_ops
    from concourse.dve_spec import Spec, Src0, Src1, C0, C1, lower, _has_src1
    from concourse.dve_uop import DveOpSpec

    name = "DUAL_SCALE_ADD_ANT"
    for o in dve_ops.OPS:
        if o.name == name:
            _DUAL = o
            return o
    spec = Spec(
        body=Src0 * C0 + Src1 * C1,
        reference=lambda in0, in1, s0, s1, imm2: (
            in0.astype(np.float32) * s0 + in1.astype(np.float32) * s1
        ),
    )
    row = max(dve_ops._SUB_OPCODE_FOR_NAME.values()) + 1
    assert row < 0x20
    dve_ops._SUB_OPCODE_FOR_NAME[name] = row
    shas = {}
    for ver in ("v3", "v4"):
        tmp = DveOpSpec(name=name, opcode=row, uops=lower(spec, ver=ver),
                        rd1_en=_has_src1(spec))
        shas[ver] = tmp.sha(ver)
    op = dve_ops.DveOp(name, spec, subdim=False, uops_sha=shas)
    dve_ops.OPS.append(op)
    dve_ops.CUSTOM_DVE_SPECS[name] = spec
    _DUAL = op
    return op


def _build(allowed):
    """Trace + compile the per-core kernel, specialized on the allowed
    shared-expert sets (from sim_domain)."""
    import concourse.bacc as bacc
    import concourse.bass as bass
    import concourse.mybir as mybir
    import concourse.tile as tile

    DUAL = _get_dual()
    EXP3M = _get_exp3m()
    f32 = mybir.dt.float32
    bf16 = mybir.dt.bfloat16
    Alu = mybir.AluOpType
    Act = mybir.ActivationFunctionType
    Ax = mybir.AxisListType

    nc = bacc.Bacc(None, target_bir_lowering=False, debug=False)

    xt = nc.declare_dram_parameter("xt", [5, DIN, BC], bf16, isOutput=False)
    wsp = nc.declare_dram_parameter("wsp", [D_NUM, DIN, N_ES * H], bf16, isOutput=False)
    wsh = nc.declare_dram_parameter("wsh", [DIN, N_SH * H], bf16, isOutput=False)
    wg = nc.declare_dram_parameter("wg", [DIN, D_NUM * 6], bf16, isOutput=False)
    wgs = nc.declare_dram_parameter("wgs", [DIN, 12], bf16, isOutput=False)
    bias4 = nc.declare_dram_parameter("bias4", [128, NBIAS], bf16, isOutput=False)
    ones128 = nc.declare_dram_parameter("ones128", [128, 128], bf16, isOutput=False)
    bmask = nc.declare_dram_parameter("bmask", [128, D_NUM, GRP, 6], f32, isOutput=False)
    out = nc.declare_dram_parameter("out", [5, BC, H], bf16, isOutput=True)

    with tile.TileContext(nc) as tc:
        with (
            tc.tile_pool(name="wpool", bufs=1) as wp,
            tc.tile_pool(name="xpool", bufs=2) as xp,
            tc.tile_pool(name="ogpool", bufs=2) as ogp,
            tc.tile_pool(name="smpool", bufs=3) as sp,
            tc.tile_pool(name="rpool", bufs=7) as rp,
            tc.tile_pool(name="scrpool", bufs=4) as scp,
            tc.tile_pool(name="pbig", bufs=3, space=bass.MemorySpace.PSUM) as pb,
            tc.tile_pool(name="pgate", bufs=1, space=bass.MemorySpace.PSUM) as pg,
        ):
            # ---- small weights first so the gate phase starts ASAP ----
            wg_sb = wp.tile([128, KC, D_NUM * 6], bf16, tag="wg")
            nc.sync.dma_start(wg_sb[:], wg.rearrange("(c p) n -> p c n", p=128))
            wgs_sb = wp.tile([128, KC, 12], bf16, tag="wgs")
            nc.sync.dma_start(wgs_sb[:], wgs.rearrange("(c p) n -> p c n", p=128))
            bias_sb = wp.tile([128, NBIAS], bf16, tag="bias")
            nc.sync.dma_start(bias_sb[:], bias4[:])
            ones_sb = wp.tile([128, 128], bf16, tag="ones")
            nc.sync.dma_start(ones_sb[:], ones128[:])
            bmask_sb = wp.tile([128, D_NUM, GRP, 6], f32, tag="bmask")
            nc.sync.dma_start(bmask_sb[:], bmask[:])
            cexp = wp.tile([128, 2], f32, tag="cexp")
            nc.vector.memset(cexp[:, 0:1], EA)
            nc.vector.memset(cexp[:, 1:2], EB)

            # x of group 0 (per-domain tiles) interleaved with expert weights
            # so domain-0 compute starts after ~1.5 MB of DMA, not ~9 MB.
            # Separate tiles per slice i -> per-domain DMA dependencies.
            wsp_sb = wp.tile([128, D_NUM, KC, N_ES * H], bf16, tag="wsp")
            wsh_sb = wp.tile([128, KC, N_SH * H], bf16, tag="wsh")

            def alloc_xtg(g):
                # xs slice (i=4) first: the gsh gate matmuls and shared experts
                # unblock early, so the group softmax isn't the critical path.
                j0 = g * (GRP * 128)
                xtg = [None] * 5
                for i in (4, 0, 1, 2, 3):
                    xi = xp.tile([128, KC, GRP * 128], bf16, tag=f"xtg{i}",
                                 name=f"xtg{g}_{i}")
                    nc.sync.dma_start(
                        xi[:], xt[i, :, j0 : j0 + GRP * 128].rearrange(
                            "(c p) j -> p c j", p=128))
                    xtg[i] = xi
                    if g == 0:
                        if i < D_NUM:
                            nc.sync.dma_start(
                                wsp_sb[:, i], wsp[i].rearrange("(c p) n -> p c n", p=128))
                        else:
                            nc.sync.dma_start(
                                wsh_sb[:], wsh.rearrange("(c p) n -> p c n", p=128))
                return xtg

            def gate_mm_emitters(g, xtg, gbank):
                """List of closures, one per gate matmul (bias first)."""
                gbd, gbs = gbank
                ems = [lambda: nc.tensor.matmul(
                    gbd[:], ones_sb[0:32, :], bias_sb[0:32, OFF_GBD : OFF_GBD + GB_GS],
                    start=True, stop=False, skip_group_check=True),
                       lambda: nc.tensor.matmul(
                    gbs[:], ones_sb[0:32, :], bias_sb[0:32, OFF_GBS : NBIAS],
                    start=True, stop=False, skip_group_check=True)]
                def dom(d, t, c):
                    o0 = (d * GRP + t) * 6
                    ems.append(lambda: nc.tensor.matmul(
                        gbd[:, o0 : o0 + 6],
                        xtg[d][:, c, t * 128 : (t + 1) * 128],
                        wg_sb[:, c, 6 * d : 6 * d + 6],
                        start=False, stop=False, skip_group_check=True))
                def gsh(t, c, last):
                    o1 = t * 12
                    ems.append(lambda: nc.tensor.matmul(
                        gbs[:, o1 : o1 + 12],
                        xtg[4][:, c, t * 128 : (t + 1) * 128],
                        wgs_sb[:, c, :],
                        start=False, stop=last, skip_group_check=True))
                if g == 0:
                    # gsh first (xs slice lands first), then domain-major
                    for t in range(GRP):
                        for c in range(KC):
                            gsh(t, c, t == GRP - 1 and c == KC - 1)
                    for d in range(D_NUM):
                        for t in range(GRP):
                            for c in range(KC):
                                dom(d, t, c)
                else:
                    for t in range(GRP):
                        for d in range(D_NUM):
                            for c in range(KC):
                                dom(d, t, c)
                        for c in range(KC):
                            gsh(t, c, t == GRP - 1 and c == KC - 1)
                return ems

            def emit_softmax(g, gbank):
                gbd, gbs = gbank
                gview = gbd.rearrange("p (d t s) -> p d t s", d=D_NUM, t=GRP)
                e1 = sp.tile([128, D_NUM, GRP, 6], f32, tag="e1", name=f"e1_{g}")
                nc.scalar.activation(e1[:], gview, Act.Exp)
                s1 = sp.tile([128, D_NUM, GRP], f32, tag="s1", name=f"s1_{g}")
                nc.vector.tensor_reduce(s1[:], e1[:], axis=Ax.X, op=Alu.add)
                r1 = sp.tile([128, D_NUM, GRP], f32, tag="r1", name=f"r1_{g}")
                nc.vector.reciprocal(r1[:], s1[:])
                gn = sp.tile([128, D_NUM, GRP, 6], f32, tag="gn", name=f"gn_{g}")
                nc.vector.tensor_tensor(
                    gn[:], e1[:], r1[:, :, :, None].to_broadcast([128, D_NUM, GRP, 6]),
                    Alu.mult)
                # second exp (input in [0,1]) fused with the mask, on Vector
                e2m = sp.tile([128, D_NUM, GRP, 6], f32, tag="e2m", name=f"e2m_{g}")
                nc.vector._custom_dve(
                    EXP3M,
                    out=e2m[:].rearrange("p a b c -> p (a b c)"),
                    in0=gn[:].rearrange("p a b c -> p (a b c)"),
                    in1=bmask_sb[:].rearrange("p a b c -> p (a b c)"),
                    s0=cexp[:, 0:1], s1=cexp[:, 1:2], imm2=EC)
                s2 = sp.tile([128, D_NUM, GRP], f32, tag="s2", name=f"s2_{g}")
                nc.vector.tensor_reduce(s2[:], e2m[:], axis=Ax.X, op=Alu.add)
                r2 = sp.tile([128, D_NUM, GRP], f32, tag="r2", name=f"r2_{g}")
                nc.vector.reciprocal(r2[:], s2[:])
                g2 = sp.tile([128, D_NUM, GRP, 6], f32, tag="g2", name=f"g2_{g}")
                nc.vector.tensor_tensor(
                    g2[:], e2m[:], r2[:, :, :, None].to_broadcast([128, D_NUM, GRP, 6]),
                    Alu.mult)

                gsview = gbs.rearrange("p (t s) -> p t s", t=GRP)
                egs = sp.tile([128, GRP, 12], f32, tag="egs", name=f"egs{g}")
                nc.scalar.activation(egs[:], gsview, Act.Exp)
                sgs = sp.tile([128, GRP], f32, tag="sgs", name=f"sgs{g}")
                nc.vector.tensor_reduce(sgs[:], egs[:], axis=Ax.X, op=Alu.add)
                rgs = sp.tile([128, GRP], f32, tag="rgs", name=f"rgs{g}")
                nc.vector.reciprocal(rgs[:], sgs[:])
                gs = sp.tile([128, GRP, 12], f32, tag="gs", name=f"gs{g}")
                nc.vector.tensor_tensor(
                    gs[:], egs[:], rgs[:, :, None].to_broadcast([128, GRP, 12]), Alu.mult)
                return g2, gs

            pending = [None]

            def flush_pending():
                if pending[0] is not None:
                    em, pending[0] = pending[0], None
                    em()

            def emit_tile(g, t, xtg, og, g2, gs, gate_block):
                """Expert matmuls + evictions + combines for one 128-row tile.
                gate_block: next-group gate-MM emitters woven into the stream."""
                j0 = g * (GRP * 128)
                gi = iter(gate_block)

                def weave(n):
                    for _ in range(n):
                        em = next(gi, None)
                        if em is not None:
                            em()

                # --- spec experts: two 2-bank psum tiles, bias via ONE 4-strip
                # volley (4 concurrent K=32 row-tiled matmuls)
                ps_pair = [
                    pb.tile([128, 1024], f32, tag="pb", name=f"ps{g}_{t}_0"),
                    pb.tile([128, 1024], f32, tag="pb", name=f"ps{g}_{t}_1"),
                ]
                for d in range(D_NUM):
                    rr = 32 * d
                    nc.tensor.matmul(ps_pair[d // 2][:, 512 * (d % 2) : 512 * (d % 2) + 512],
                                     ones_sb[rr : rr + 32, :],
                                     bias_sb[rr : rr + 32, 0:512],
                                     start=True, stop=False, skip_group_check=True,
                                     tile_position=(rr, 0))
                for half in range(2):
                    ps = ps_pair[half]
                    for dd in range(2):
                        d = 2 * half + dd
                        o = 512 * dd
                        for c in range(KC):
                            nc.tensor.matmul(ps[:, o : o + 512],
                                             xtg[d][:, c, t * 128 : (t + 1) * 128],
                                             wsp_sb[:, d, c, :], start=False,
                                             stop=(c == KC - 1), skip_group_check=True)
                            weave(1)

                # --- shared experts: one 2-bank psum tile
                ps_s = pb.tile([128, 1024], f32, tag="pb", name=f"pss{g}_{t}")
                nc.tensor.matmul(ps_s[:, 0:512], ones_sb[0:32, :],
                                 bias_sb[0:32, OFF_SHB : OFF_SHB + 512],
                                 start=True, stop=False, skip_group_check=True,
                                 tile_position=(0, 0))
                nc.tensor.matmul(ps_s[:, 512:1024], ones_sb[32:64, :],
                                 bias_sb[32:64, OFF_SHB : OFF_SHB + 512],
                                 start=True, stop=False, skip_group_check=True,
                                 tile_position=(32, 0))
                for c in range(KC):
                    nc.tensor.matmul(ps_s[:, 0:512], xtg[4][:, c, t * 128 : (t + 1) * 128],
                                     wsh_sb[:, c, 0:512], start=False,
                                     stop=(c == KC - 1), skip_group_check=True)
                    nc.tensor.matmul(ps_s[:, 512:1024], xtg[4][:, c, t * 128 : (t + 1) * 128],
                                     wsh_sb[:, c, 512:1024], start=False,
                                     stop=(c == KC - 1), skip_group_check=True)
                    weave(1)
                weave(len(gate_block))  # flush leftovers

                # --- evictions: ONE wide relu act per psum tile -> bf16 SBUF
                r01 = rp.tile([128, 1024], bf16, tag="r", name=f"r01_{g}_{t}")
                nc.scalar.activation(r01[:], ps_pair[0][:], Act.Relu)
                r23 = rp.tile([128, 1024], bf16, tag="r", name=f"r23_{g}_{t}")
                nc.scalar.activation(r23[:], ps_pair[1][:], Act.Relu)
                rsh = rp.tile([128, 1024], bf16, tag="r", name=f"rsh_{g}_{t}")
                nc.scalar.activation(rsh[:], ps_s[:], Act.Relu)
                # previous tile's ScalarE products run AFTER this tile's
                # evictions so PSUM banks always turn over promptly
                flush_pending()

                def rspec(d, e):
                    rt = r01 if d < 2 else r23
                    o = (d % 2) * 512 + e * 256
                    return rt[:, o : o + 256]

                def rshared(s):
                    return rsh[:, s * 256 : (s + 1) * 256]

                # --- combine: DUAL_SCALE_ADD fuses two gated terms per Vector
                # instruction; GpSimd does the tensor_tensor glue adds.
                def dual(out_ap, a_src, a_scale, b_src, b_scale):
                    nc.vector._custom_dve(DUAL, out=out_ap, in0=a_src, in1=b_src,
                                          s0=a_scale, s1=b_scale)

                scr = scp.tile([128, 6, 256], bf16, tag="scr", name=f"scr_{g}_{t}")
                for d in range(D_NUM):
                    og_d = og[:, d, t, :]
                    dual(og_d, rspec(d, 0), g2[:, d, t, 0:1],
                         rspec(d, 1), g2[:, d, t, 1:2])
                    al = allowed[d]
                    i = 0
                    while i + 1 < len(al):
                        sa, sb = al[i], al[i + 1]
                        dual(scr[:, 4], rshared(sa), g2[:, d, t, 2 + sa : 3 + sa],
                             rshared(sb), g2[:, d, t, 2 + sb : 3 + sb])
                        if d < 2:
                            nc.vector.tensor_tensor(og_d, og_d, scr[:, 4], Alu.add)
                        else:
                            nc.gpsimd.tensor_tensor(og_d, og_d, scr[:, 4], Alu.add)
                        i += 2
                    if i < len(al):
                        s = al[i]
                        nc.vector.scalar_tensor_tensor(
                            og_d, rshared(s), g2[:, d, t, 2 + s : 3 + s], og_d,
                            Alu.mult, Alu.add)

                # og_s: 12 terms as 6 duals; chain A accumulates on Vector,
                # chain B glue on GpSimd, merged by GpSimd into the og slot.
                og_s = og[:, 4, t, :]
                dual(og_s, rspec(0, 0), gs[:, t, 0:1], rspec(0, 1), gs[:, t, 1:2])
                dual(scr[:, 0], rspec(1, 0), gs[:, t, 2:3], rspec(1, 1), gs[:, t, 3:4])
                dual(scr[:, 1], rshared(0), gs[:, t, 8:9], rshared(1), gs[:, t, 9:10])
                nc.vector.tensor_tensor(og_s, og_s, scr[:, 0], Alu.add)
                nc.vector.tensor_tensor(og_s, og_s, scr[:, 1], Alu.add)
                # chain B (deferred one tile): six ScalarE scaled-copy products
                # + GpSimd/Vector glue, then the output DMA for this tile.
                def chain_b():
                    sc2 = scp.tile([128, 6, 256], bf16, tag="sc2", name=f"sc2_{g}_{t}")
                    for i, (src, k) in enumerate(
                            ((rspec(2, 0), 4), (rspec(2, 1), 5), (rspec(3, 0), 6),
                             (rspec(3, 1), 7), (rshared(2), 10), (rshared(3), 11))):
                        nc.scalar.activation(sc2[:, i], src, Act.Copy,
                                             scale=gs[:, t, k : k + 1])
                    nc.gpsimd.tensor_tensor(sc2[:, 0], sc2[:, 0], sc2[:, 1], Alu.add)
                    nc.gpsimd.tensor_tensor(sc2[:, 2], sc2[:, 2], sc2[:, 3], Alu.add)
                    nc.gpsimd.tensor_tensor(sc2[:, 4], sc2[:, 4], sc2[:, 5], Alu.add)
                    nc.vector.tensor_tensor(sc2[:, 0], sc2[:, 0], sc2[:, 2], Alu.add)
                    nc.vector.tensor_tensor(sc2[:, 0], sc2[:, 0], sc2[:, 4], Alu.add)
                    nc.vector.tensor_tensor(og_s, og_s, sc2[:, 0], Alu.add)

                    r0_ = j0 + t * 128
                    nc.sync.dma_start(
                        out[:, r0_ : r0_ + 128, :].rearrange("i p h -> p i h"),
                        og[:, :, t, :])

                pending[0] = chain_b

            # ---- software pipeline over groups ----
            xtg_cur = alloc_xtg(0)
            gb0 = pg.tile([128, NGB], f32, tag="pg", name="gb0")
            gbank_cur = (gb0[:, 0:GB_GS], gb0[:, GB_GS:NGB])
            for em in gate_mm_emitters(0, xtg_cur, gbank_cur):
                em()
            sm_cur = emit_softmax(0, gbank_cur)

            for g in range(NG):
                og = ogp.tile([128, 5, GRP, H], bf16, tag="og", name=f"og{g}")
                if g + 1 < NG:
                    xtg_next = alloc_xtg(g + 1)
                    gbn = pg.tile([128, NGB], f32, tag="pg", name=f"gb{g+1}")
                    gbank_next = (gbn[:, 0:GB_GS], gbn[:, GB_GS:NGB])
                    ems = gate_mm_emitters(g + 1, xtg_next, gbank_next)
                    # split gate MMs across this group's tiles (skip tile 0 so the
                    # next group's xtg DMA has time to land)
                    nblk = GRP - 1
                    per = (len(ems) + nblk - 1) // nblk
                    blocks = [[]] + [ems[i * per : (i + 1) * per] for i in range(nblk)]
                else:
                    blocks = [[] for _ in range(GRP)]
                for t in range(GRP):
                    emit_tile(g, t, xtg_cur, og, sm_cur[0], sm_cur[1], blocks[t])
                if g + 1 < NG:
                    sm_cur = emit_softmax(g + 1, gbank_next)
                    xtg_cur = xtg_next
            flush_pending()

    nc.compile()
    return nc


def _prep_inputs(inputs):
    """Host-side shard + relayout. Returns (in_maps, allowed)."""
    import ml_dtypes
    bf16_np = ml_dtypes.bfloat16

    x_list = np.asarray(inputs["x_list"], dtype=np.float32)
    sim_domain = np.asarray(inputs["sim_domain"])
    W_spec = np.asarray(inputs["W_spec"], dtype=np.float32)
    b_spec = np.asarray(inputs["b_spec"], dtype=np.float32)
    W_sh = np.asarray(inputs["W_sh"], dtype=np.float32)
    b_sh = np.asarray(inputs["b_sh"], dtype=np.float32)
    W_gate = np.asarray(inputs["W_gate"], dtype=np.float32)
    b_gate = np.asarray(inputs["b_gate"], dtype=np.float32)
    W_gate_sh = np.asarray(inputs["W_gate_sh"], dtype=np.float32)
    b_gate_sh = np.asarray(inputs["b_gate_sh"], dtype=np.float32)

    mem = (sim_domain[:, :, None] == np.arange(D_NUM)[None, None, :]).any(axis=1)
    allowed = tuple(tuple(int(s) for s in range(N_SH) if mem[d, s]) for d in range(D_NUM))

    wsp = np.ascontiguousarray(
        W_spec.transpose(0, 2, 1, 3).reshape(D_NUM, DIN, N_ES * H)
    ).astype(bf16_np)
    wsh = np.ascontiguousarray(W_sh.transpose(1, 0, 2).reshape(DIN, N_SH * H)).astype(bf16_np)
    wg = np.ascontiguousarray(W_gate.transpose(1, 0, 2).reshape(DIN, D_NUM * 6)).astype(bf16_np)
    wgs = np.ascontiguousarray(W_gate_sh).astype(bf16_np)

    bias4 = np.zeros((128, NBIAS), np.float32)
    for d in range(D_NUM):
        bias4[32 * d, 0:512] = b_spec[d].reshape(512)
    bias4[0, OFF_SHB : OFF_SHB + 512] = b_sh[0:2].reshape(512)
    bias4[32, OFF_SHB : OFF_SHB + 512] = b_sh[2:4].reshape(512)
    bias4[0, OFF_GBD : OFF_GBD + GB_GS] = np.repeat(
        b_gate[:, None, :], GRP, axis=1).reshape(-1)
    bias4[0, OFF_GBS:NBIAS] = np.tile(b_gate_sh, GRP)
    bias4 = bias4.astype(bf16_np)

    ones128 = np.zeros((128, 128), np.float32)
    for s in range(4):
        ones128[32 * s] = 1.0
    ones128 = ones128.astype(bf16_np)

    bmask_row = np.ones((D_NUM, 6), np.float32)
    bmask_row[:, N_ES:] = mem.astype(np.float32)
    bmask = np.broadcast_to(
        np.repeat(bmask_row[None, :, None, :], GRP, axis=2), (128, D_NUM, GRP, 6)
    ).copy()

    shared = {"wsp": wsp, "wsh": wsh, "wg": wg, "wgs": wgs,
              "bias4": bias4, "ones128": ones128, "bmask": bmask}
    in_maps = []
    for c in range(N_CORES):
        sl = x_list[:, c * BC : (c + 1) * BC, :]
        xt_c = np.ascontiguousarray(sl.transpose(0, 2, 1)).astype(bf16_np)
        in_maps.append({"xt": xt_c, **shared})
    return in_maps, allowed


def _run(inputs, trace=False, trace_kwargs=None):
    from concourse.bass_utils import run_bass_kernel_spmd

    in_maps, allowed = _prep_inputs(inputs)
    key = allowed
    if key not in _BUILD_CACHE:
        _BUILD_CACHE[key] = _build(allowed)
    nc = _BUILD_CACHE[key]

    kw = {}
    if trace:
        kw["trace"] = True
        if trace_kwargs:
            kw.update(trace_kwargs)
    res = run_bass_kernel_spmd(nc, in_maps, list(range(N_CORES)), **kw)
    full = np.empty((5, B, H), np.float32)
    for c in range(N_CORES):
        full[:, c * BC : (c + 1) * BC, :] = np.asarray(
            res.results[c]["out"], dtype=np.float32)
    return full, res


def kernel(**inputs):
    full, _ = _run(inputs)
    return full
